# revision 2
# baseline (speedup 1.0000x reference)
"""Trainium2 Bass kernel v2 for a dense transformer block (pre-LN, causal MHA
+ GELU MLP).  Data-parallel over batch: B == 8 == n_cores, one batch element
per NeuronCore, no collectives.

What changed vs the v1 baseline (measured on this axon-tunneled TRN2 pod):
  - f32r matmuls run at fp32 rate (~850 ns per N=512 matmul, 4 cyc/col); all
    matmul operands are now bf16 (1 cyc/col), halving weight bytes too.
  - DMA cost here is dominated by a large fixed per-dma_start cost, so the
    305 DMAs of v1 are collapsed into ~15 big contiguous transfers of
    host-packed weights, split across the two HWDGE rings (sync + scalar).
  - Weights are loaded exactly once (v1 streamed w1/w2 twice).
  - Attention keeps per-head [64, T] tiles so the softmax-normalize multiply
    lands directly in the tile (v1 DMA'd the odd head across partitions).
  - Softmax denominator reciprocal is partition-broadcast with a K=1 matmul
    on the PE (v1 used 32 gpsimd SWDGE broadcast DMAs).
  - Causal masking uses subrange matmuls instead of zero-filling masked
    blocks (25% fewer score/attnV cycles).

Per-core layout strategy is otherwise the same as v1: LN in token layout via
bn_stats, PE-transpose to e-partition layout for the matmuls, attention
computed transposed with an appended ones-column on V producing the softmax
denominator, MLP split into four FF quarters so weights stream through SBUF.
"""

import numpy as np

B, T, E = 8, 1024, 1024
NH, HD, FF = 16, 64, 4096
NPAIR = NH // 2
EPS = 1e-5
NCORES = 8
TCH = T // 128           # 8 token chunks
ECH = E // 128           # 8 embedding chunks
FCH = FF // 128          # 32 mlp hidden chunks
FHALF = 4                # mlp FF chunks (weight streaming granularity)
FCH_H = FCH // FHALF     # 8 hidden chunks per streaming chunk
TQW = 512
NTQ = T // TQW           # 2

_STAGES = {"ln": 1, "qkv": 2, "attn": 3, "proj": 4, "full": 5}


def _build_program(flags, stage="full"):
    import concourse.bass as bass
    import concourse.tile as tile
    from concourse import bacc, mybir
    from concourse.masks import make_identity, make_upper_triangular

    sn = _STAGES[stage]
    f32 = mybir.dt.float32
    bf16 = mybir.dt.bfloat16
    AF = mybir.ActivationFunctionType

    nc = bacc.Bacc("TRN2", target_bir_lowering=False, debug=False,
                   num_devices=NCORES)

    x_d = nc.dram_tensor("x", [T, E], f32, kind="ExternalInput").ap()
    wqT_d = nc.dram_tensor("wqT", [128, ECH, E], bf16,
                           kind="ExternalInput").ap()
    wkT_d = nc.dram_tensor("wkT", [128, ECH, E], bf16,
                           kind="ExternalInput").ap()
    wvT_d = nc.dram_tensor("wvT", [128, ECH, E], bf16,
                           kind="ExternalInput").ap()
    wpT_d = nc.dram_tensor("wpT", [HD, NH, E], bf16,
                           kind="ExternalInput").ap()
    w1T_d = nc.dram_tensor("w1T", [FHALF, 128, ECH, FF // FHALF], bf16,
                           kind="ExternalInput").ap()
    w2T_d = nc.dram_tensor("w2T", [FHALF, 128, FCH_H, E], bf16,
                           kind="ExternalInput").ap()
    b1c_d = nc.dram_tensor("b1c", [128, FCH], f32, kind="ExternalInput").ap()
    ln1g_d = ln1b_d = ln2g_d = ln2b_d = bp_d = b2_d = None
    if flags["ln1_gb"]:
        ln1g_d = nc.dram_tensor("ln1_g", [E], f32, kind="ExternalInput").ap()
        ln1b_d = nc.dram_tensor("ln1_b", [E], f32, kind="ExternalInput").ap()
    if flags["ln2_gb"]:
        ln2g_d = nc.dram_tensor("ln2_g", [E], f32, kind="ExternalInput").ap()
        ln2b_d = nc.dram_tensor("ln2_b", [E], f32, kind="ExternalInput").ap()
    if flags["b_proj"]:
        bp_d = nc.dram_tensor("b_proj", [E], f32, kind="ExternalInput").ap()
    if flags["b2"]:
        b2_d = nc.dram_tensor("b2", [E], f32, kind="ExternalInput").ap()
    out_d = nc.dram_tensor("out", [T, E], f32, kind="ExternalOutput").ap()

    dbg_outs = {}

    def dbg_tensor(name, shape):
        dbg_outs[name] = nc.dram_tensor(name, shape, f32,
                                        kind="ExternalOutput").ap()
        return dbg_outs[name]

    with tile.TileContext(nc) as tc:
        with (
            tc.tile_pool(name="const", bufs=1) as p_const,
            tc.tile_pool(name="resid", bufs=1) as p_resid,
            tc.tile_pool(name="attn", bufs=NH) as p_attn,
            tc.tile_pool(name="ps", bufs=7, space="PSUM") as p_ps,
        ):
            # ---- constants ----
            ident = p_const.tile([128, 128], bf16, tag="ident", name="ident")
            make_identity(nc, ident[:])
            tri_f = p_const.tile([128, 128], f32, tag="trif", name="trif")
            make_upper_triangular(nc, tri_f[:], val=1.0, diag=True)
            tri = p_const.tile([128, 128], bf16, tag="tri", name="tri")
            nc.vector.tensor_copy(tri[:], tri_f[:])
            epst = p_const.tile([128, 1], f32, tag="epst", name="epst")
            nc.vector.memset(epst[:], EPS)
            onesb = p_const.tile([128, HD], bf16, tag="onesb", name="onesb")
            nc.vector.memset(onesb[:], 1.0)
            b1c = p_const.tile([128, FCH], f32, tag="b1c", name="b1c")
            nc.scalar.dma_start(b1c[:], b1c_d[:, :])

            def bcast_row(dram_vec, tag):
                tf = p_const.tile([128, E], f32, tag=tag + "f", name=tag + "f")
                src = bass.AP(tensor=dram_vec.tensor, offset=dram_vec.offset,
                              ap=[[0, 128]] + list(dram_vec.ap))
                nc.gpsimd.dma_start(tf[:], src)
                tb = p_const.tile([128, E], bf16, tag=tag, name=tag)
                nc.vector.tensor_copy(tb[:], tf[:])
                return tf, tb

            ln1g_b = bcast_row(ln1g_d, "ln1g")[1] if flags["ln1_gb"] else None
            ln1b_b = bcast_row(ln1b_d, "ln1b")[1] if flags["ln1_gb"] else None
            ln2g_b = bcast_row(ln2g_d, "ln2g")[1] if flags["ln2_gb"] else None
            ln2b_b = bcast_row(ln2b_d, "ln2b")[1] if flags["ln2_gb"] else None
            bp_b = bcast_row(bp_d, "bpb")[0] if flags["b_proj"] else None
            b2_b = bcast_row(b2_d, "b2b")[0] if flags["b2"] else None

            # ---- residual: single [128, TCH, E] f32 tile, one DMA each way
            xt = p_resid.tile([128, TCH, E], f32, tag="resid", name="resid")
            nc.sync.dma_start(xt[:], x_d.rearrange("(c p) e -> p c e", p=128))

            # ---- layernorm in token layout + PE transpose to [E, T] ----
            def layer_norm_transposed(g_b, b_b, p_ht, p_htok, p_small):
                ht = [p_ht.tile([128, T], bf16, tag="ht", name="ht")
                      for _ in range(ECH)]
                for tch in range(TCH):
                    xti = xt[:, tch, :]
                    st = p_small.tile([128, 2, 6], f32, tag="st", name="st")
                    nc.vector.bn_stats(st[:, 0, :], xti[:, 0:512])
                    nc.vector.bn_stats(st[:, 1, :], xti[:, 512:1024])
                    mv = p_small.tile([128, 2], f32, tag="mv", name="mv")
                    nc.vector.bn_aggr(mv[:], st[:])
                    sq = p_small.tile([128, 1], f32, tag="sq", name="sq")
                    nc.scalar.activation(sq[:], mv[:, 1:2], AF.Sqrt,
                                         bias=epst[:])
                    rsig = p_small.tile([128, 1], f32, tag="rsig", name="rsig")
                    nc.vector.reciprocal(rsig[:], sq[:])
                    h = p_htok.tile([128, E], bf16, tag="htok", name="htok")
                    nc.vector.tensor_scalar(h[:], xti, mv[:, 0:1],
                                            rsig[:], mybir.AluOpType.subtract,
                                            mybir.AluOpType.mult)
                    if g_b is not None:
                        nc.vector.tensor_mul(h[:], h[:], g_b[:])
                        nc.vector.tensor_add(h[:], h[:], b_b[:])
                    for ec in range(ECH):
                        pst = p_ps.tile([128, 512], bf16, tag="ps", name="ps")
                        with nc.allow_low_precision(
                                reason="bf16 transpose of bf16 data"):
                            nc.tensor.transpose(pst[:, 0:128],
                                                h[:, 128 * ec:128 * (ec + 1)],
                                                ident[:])
                        nc.vector.tensor_copy(
                            ht[ec][:, 128 * tch:128 * (tch + 1)],
                            pst[:, 0:128])
                return ht

            # Pool enter order is LIFO-constrained: longest-lived first.
            # -- pool group B: vt + qk (closed after attention) --
            b_cms = [
                tc.tile_pool(name="vpool", bufs=TCH),
                tc.tile_pool(name="qk", bufs=4),
            ]
            p_v, p_qk = (cm.__enter__() for cm in b_cms)
            att_cms = [
                tc.tile_pool(name="esc", bufs=6),
                tc.tile_pool(name="norm", bufs=4),
            ]
            p_esc, p_norm = (cm.__enter__() for cm in att_cms)
            # -- pool group A: QKV weights + ht (closed after attention) --
            a_cms = [
                tc.tile_pool(name="wqkv", bufs=1),
                tc.tile_pool(name="ht", bufs=ECH),
                tc.tile_pool(name="htok", bufs=3),
                tc.tile_pool(name="small", bufs=6),
            ]
            p_w, p_ht, p_htok, p_small = (cm.__enter__() for cm in a_cms)

            # weight loads, split across the two HWDGE rings
            wvT_t = p_w.tile([128, ECH, E], bf16, tag="wv", name="wv")
            nc.sync.dma_start(wvT_t[:], wvT_d[:, :, :])
            wqT_t = p_w.tile([128, ECH, E], bf16, tag="wq", name="wq")
            nc.scalar.dma_start(wqT_t[:], wqT_d[:, :, :])
            wkT_t = p_w.tile([128, ECH, E], bf16, tag="wk", name="wk")
            nc.sync.dma_start(wkT_t[:], wkT_d[:, :, :])

            ht = layer_norm_transposed(ln1g_b, ln1b_b, p_ht, p_htok, p_small)

            if sn == 1:
                o = dbg_tensor("dbg_ht", [E, T])
                for ec in range(ECH):
                    ob = p_htok.tile([128, T], f32, tag="dbgcast",
                                     name="dbgcast")
                    nc.vector.tensor_copy(ob[:], ht[ec][:])
                    nc.sync.dma_start(o[128 * ec:128 * (ec + 1), :], ob[:])

            # ---- V = h @ wv -> token layout [t, head, 65] + ones col ----
            vt = []
            for tch in range(TCH):
                v = p_v.tile([128, NH, HD + 1], bf16, tag="v", name="v")
                nc.vector.memset(v[:, :, HD:HD + 1], 1.0)
                vt.append(v)
            for half in range(2):
                esl = slice(512 * half, 512 * (half + 1))
                for tch in range(TCH):
                    ps = p_ps.tile([128, 512], f32, tag="ps", name="ps")
                    for ec in range(ECH):
                        nc.tensor.matmul(
                            ps[:], ht[ec][:, 128 * tch:128 * (tch + 1)],
                            wvT_t[:, ec, esl], start=(ec == 0),
                            stop=(ec == ECH - 1))
                    nc.vector.tensor_copy(
                        vt[tch][:, 8 * half:8 * (half + 1), 0:HD],
                        ps[:].rearrange("p (h d) -> p h d", d=HD))

            # ---- per pair: Q/K, then attention for its two heads ----
            attn_h = [None] * NH
            for pair in range(NPAIR if sn >= 2 else 0):
                cols = slice(128 * pair, 128 * (pair + 1))
                qT = p_qk.tile([128, T], bf16, tag="qk", name="qk")
                kT = p_qk.tile([128, T], bf16, tag="qk", name="qk")
                for (w_t, dst) in ((wqT_t, qT), (wkT_t, kT)):
                    for th in range(NTQ):
                        tsl = slice(TQW * th, TQW * (th + 1))
                        ps = p_ps.tile([128, 512], f32, tag="ps", name="ps")
                        for ec in range(ECH):
                            nc.tensor.matmul(
                                ps[:], w_t[:, ec, cols], ht[ec][:, tsl],
                                start=(ec == 0), stop=(ec == ECH - 1))
                        nc.vector.tensor_copy(dst[:, tsl], ps[:])

                if sn == 2 and pair == 0:
                    oq = dbg_tensor("dbg_qT", [128, T])
                    qf = p_htok.tile([128, T], f32, tag="dbgq", name="dbgq")
                    nc.vector.tensor_copy(qf[:], qT[:])
                    nc.sync.dma_start(oq[:, :], qf[:])
                    ok_ = dbg_tensor("dbg_kT", [128, T])
                    kf = p_htok.tile([128, T], f32, tag="dbgq", name="dbgq")
                    nc.vector.tensor_copy(kf[:], kT[:])
                    nc.sync.dma_start(ok_[:, :], kf[:])
                    o2 = dbg_tensor("dbg_v", [T, NH * (HD + 1)])
                    for tch in range(TCH):
                        vf = p_htok.tile([128, NH * (HD + 1)], f32,
                                         tag="dbgv", name="dbgv")
                        nc.vector.tensor_copy(
                            vf[:], vt[tch][:].rearrange("p h d -> p (h d)"))
                        nc.sync.dma_start(o2[128 * tch:128 * (tch + 1), :],
                                          vf[:])
                    break

                if sn < 3:
                    break

                for hp in range(2):
                    h = 2 * pair + hp
                    rows = slice(HD * hp, HD * (hp + 1))
                    qh, kh = qT[rows, :], kT[rows, :]
                    att = p_attn.tile([HD, T], bf16, tag="attn", name="attn")
                    attn_h[h] = att
                    for bq in range(NTQ):
                        qsl = slice(TQW * bq, TQW * (bq + 1))
                        nbk = min(TCH, 4 * bq + 4)
                        ps_a = p_ps.tile([128, 512], f32, tag="ps", name="ps")
                        for bk in range(nbk):
                            d = bk - 4 * bq
                            col0 = 128 * d if d > 0 else 0
                            ps_s = p_ps.tile([128, 512], f32, tag="ps",
                                             name="ps")
                            nc.tensor.matmul(
                                ps_s[:, col0:512],
                                kh[:, 128 * bk:128 * (bk + 1)],
                                qh[:, TQW * bq + col0:TQW * (bq + 1)],
                                start=True, stop=True)
                            et = p_esc.tile([128, 512], bf16, tag="esc",
                                            name="esc")
                            nc.scalar.activation(et[:, col0:512],
                                                 ps_s[:, col0:512],
                                                 AF.Exp, scale=0.125)
                            if d >= 0:
                                dsl = slice(col0, col0 + 128)
                                nc.vector.tensor_mul(et[:, dsl], et[:, dsl],
                                                     tri[:])
                            nc.tensor.matmul(
                                ps_a[0:HD + 1, col0:512], vt[bk][:, h, :],
                                et[:, col0:512], start=(bk == 0),
                                stop=(bk == nbk - 1))
                        # normalize: rcp of denominator, PE-broadcast, mul
                        rcp = p_norm.tile([HD + 1, 512], bf16, tag="rcp",
                                          name="rcp")
                        with nc.allow_low_precision(
                                reason="bf16 softmax denom reciprocal"):
                            nc.vector.reciprocal(rcp[HD:HD + 1, :],
                                                 ps_a[HD:HD + 1, :])
                        ps_b = p_ps.tile([128, 512], f32, tag="ps", name="ps")
                        nc.tensor.matmul(ps_b[0:HD, :], onesb[HD:HD + 1, :],
                                         rcp[HD:HD + 1, :], start=True,
                                         stop=True)
                        bct = p_norm.tile([HD, 512], f32, tag="bct",
                                          name="bct")
                        nc.vector.tensor_copy(bct[:], ps_b[0:HD, :])
                        nc.vector.tensor_mul(att[:, qsl], ps_a[0:HD, :],
                                             bct[:])

            # free QKV weights, ht, qk, vt, attention transients
            for cm in reversed(a_cms):
                cm.__exit__(None, None, None)
            for cm in reversed(att_cms):
                cm.__exit__(None, None, None)
            for cm in reversed(b_cms):
                cm.__exit__(None, None, None)

            if sn == 3:
                o = dbg_tensor("dbg_attnT", [NH * HD, T])
                with tc.tile_pool(name="dbga", bufs=2) as p_dbg:
                    for h in range(NH):
                        af = p_dbg.tile([HD, T], f32, tag="af", name="af")
                        nc.vector.tensor_copy(af[:], attn_h[h][:])
                        nc.sync.dma_start(o[HD * h:HD * (h + 1), :], af[:])

            # MLP + proj weight pools (w1/w2 stream in quarters; wp on top
            # of the stack so it can be released right after proj)
            w1_cm = tc.tile_pool(name="w1", bufs=2)
            p_w1 = w1_cm.__enter__()
            w2_cm = tc.tile_pool(name="w2", bufs=2)
            p_w2 = w2_cm.__enter__()
            wp_cm = tc.tile_pool(name="wp", bufs=1)
            p_wp = wp_cm.__enter__()
            wpT_t = p_wp.tile([HD, NH, E], bf16, tag="wp", name="wp")
            nc.scalar.dma_start(wpT_t[:], wpT_d[:, :, :])

            def load_w1(fh):
                t = p_w1.tile([128, ECH, FF // FHALF], bf16, tag="w1",
                              name="w1")
                (nc.sync if fh % 2 == 0 else nc.scalar).dma_start(
                    t[:], w1T_d[fh, :, :, :])
                return t

            def load_w2(fh):
                t = p_w2.tile([128, FCH_H, E], bf16, tag="w2", name="w2")
                (nc.scalar if fh % 2 == 0 else nc.sync).dma_start(
                    t[:], w2T_d[fh, :, :, :])
                return t

            w1h = [load_w1(0)]
            w2h = [load_w2(0)]

            # ---- out = x + attn @ w_proj (+ b_proj), in-place xt ----
            if sn >= 4:
                for eo in range(2):
                    esl = slice(512 * eo, 512 * (eo + 1))
                    for tch in range(TCH):
                        ps = p_ps.tile([128, 512], f32, tag="ps", name="ps")
                        for h in range(NH):
                            nc.tensor.matmul(
                                ps[:],
                                attn_h[h][:, 128 * tch:128 * (tch + 1)],
                                wpT_t[:, h, esl], start=(h == 0),
                                stop=(h == NH - 1))
                        nc.vector.tensor_add(xt[:, tch, esl], ps[:],
                                             xt[:, tch, esl])
                        if flags["b_proj"]:
                            nc.vector.tensor_add(xt[:, tch, esl],
                                                 xt[:, tch, esl],
                                                 bp_b[:, esl])

            wp_cm.__exit__(None, None, None)

            if sn == 4:
                o = dbg_tensor("dbg_out1", [T, E])
                nc.sync.dma_start(o.rearrange("(c p) e -> p c e", p=128),
                                  xt[:])

            if sn >= 5:
                # ---- LN2 + transpose ----
                mlp_cms = [
                    tc.tile_pool(name="h2t", bufs=ECH),
                    tc.tile_pool(name="htok2", bufs=3),
                    tc.tile_pool(name="small2", bufs=6),
                    tc.tile_pool(name="gt", bufs=FCH_H),
                ]
                p_h2t, p_htok2, p_small2, p_gt = (
                    cm.__enter__() for cm in mlp_cms)
                h2t = layer_norm_transposed(ln2g_b, ln2b_b, p_h2t, p_htok2,
                                            p_small2)

                # ---- MLP: u = w1^T h2T; g = gelu(u+b1); xt += g^T w2 ----
                for fh in range(FHALF):
                    if fh + 1 < FHALF:
                        w1h.append(load_w1(fh + 1))
                        w2h.append(load_w2(fh + 1))
                    gt = []
                    for fl in range(FCH_H):
                        fc = FCH_H * fh + fl
                        g = p_gt.tile([128, T], bf16, tag="gt", name="gt")
                        gt.append(g)
                        for th in range(NTQ):
                            tsl = slice(TQW * th, TQW * (th + 1))
                            ps = p_ps.tile([128, 512], f32, tag="ps",
                                           name="ps")
                            for ec in range(ECH):
                                nc.tensor.matmul(
                                    ps[:],
                                    w1h[fh][:, ec, 128 * fl:128 * (fl + 1)],
                                    h2t[ec][:, tsl], start=(ec == 0),
                                    stop=(ec == ECH - 1))
                            nc.scalar.activation(g[:, tsl], ps[:], AF.Gelu,
                                                 bias=b1c[:, fc:fc + 1])
                    for eo in range(2):
                        esl = slice(512 * eo, 512 * (eo + 1))
                        for tl in range(TCH):
                            ps = p_ps.tile([128, 512], f32, tag="ps",
                                           name="ps")
                            for fl in range(FCH_H):
                                nc.tensor.matmul(
                                    ps[:],
                                    gt[fl][:, 128 * tl:128 * (tl + 1)],
                                    w2h[fh][:, fl, esl], start=(fl == 0),
                                    stop=(fl == FCH_H - 1))
                            nc.vector.tensor_add(xt[:, tl, esl], ps[:],
                                                 xt[:, tl, esl])
                            if flags["b2"] and fh == FHALF - 1:
                                nc.vector.tensor_add(xt[:, tl, esl],
                                                     xt[:, tl, esl],
                                                     b2_b[:, esl])
                for cm in reversed(mlp_cms):
                    cm.__exit__(None, None, None)

                nc.scalar.dma_start(
                    out_d.rearrange("(c p) e -> p c e", p=128), xt[:])

            w2_cm.__exit__(None, None, None)
            w1_cm.__exit__(None, None, None)

    nc.compile()
    return nc, dbg_outs


_CACHE = {}


def _get_program(flags_key, stage="full"):
    key = (flags_key, stage)
    if key not in _CACHE:
        flags = dict(zip(("ln1_gb", "ln2_gb", "b_proj", "b2"), flags_key))
        _CACHE[key] = _build_program(flags, stage)
    return _CACHE[key]


def _flags_for(inputs):
    return {
        "ln1_gb": not (np.all(np.asarray(inputs["ln1_g"]) == 1.0)
                       and np.all(np.asarray(inputs["ln1_b"]) == 0.0)),
        "ln2_gb": not (np.all(np.asarray(inputs["ln2_g"]) == 1.0)
                       and np.all(np.asarray(inputs["ln2_b"]) == 0.0)),
        "b_proj": not np.all(np.asarray(inputs["b_proj"]) == 0.0),
        "b2": not np.all(np.asarray(inputs["b2"]) == 0.0),
    }


def _pack_weights(inputs, flags):
    import ml_dtypes
    bf16 = ml_dtypes.bfloat16
    f32 = np.float32

    def a(name):
        return np.asarray(inputs[name], f32)

    common = {}
    common["wqT"] = np.ascontiguousarray(
        a("wq").reshape(ECH, 128, E).transpose(1, 0, 2)).astype(bf16)
    common["wkT"] = np.ascontiguousarray(
        a("wk").reshape(ECH, 128, E).transpose(1, 0, 2)).astype(bf16)
    common["wvT"] = np.ascontiguousarray(
        a("wv").reshape(ECH, 128, E).transpose(1, 0, 2)).astype(bf16)
    common["wpT"] = np.ascontiguousarray(
        a("w_proj").reshape(NH, HD, E).transpose(1, 0, 2)).astype(bf16)
    w1 = a("w1")
    fw = FF // FHALF
    common["w1T"] = np.ascontiguousarray(np.stack([
        w1[:, fh * fw:(fh + 1) * fw].reshape(ECH, 128, fw).transpose(1, 0, 2)
        for fh in range(FHALF)])).astype(bf16)
    w2 = a("w2")
    common["w2T"] = np.ascontiguousarray(np.stack([
        w2[fh * fw:(fh + 1) * fw, :].reshape(FCH_H, 128, E).transpose(1, 0, 2)
        for fh in range(FHALF)])).astype(bf16)
    common["b1c"] = np.ascontiguousarray(
        a("b1").reshape(FCH, 128).transpose(1, 0)).astype(f32)
    for name, flag in (("ln1_g", "ln1_gb"), ("ln1_b", "ln1_gb"),
                       ("ln2_g", "ln2_gb"), ("ln2_b", "ln2_gb"),
                       ("b_proj", "b_proj"), ("b2", "b2")):
        if flags[flag]:
            common[name] = np.ascontiguousarray(inputs[name], f32)
    return common


def _make_in_maps(inputs, flags, cores):
    common = _pack_weights(inputs, flags)
    x = np.ascontiguousarray(inputs["x"], np.float32)
    return [{"x": x[c], **common} for c in cores]


def _run(inputs, stage="full", cores=None):
    from concourse.bass_utils import run_bass_kernel_spmd

    if cores is None:
        cores = list(range(NCORES))
    flags = _flags_for(inputs)
    flags_key = tuple(flags[k] for k in ("ln1_gb", "ln2_gb", "b_proj", "b2"))
    nc, dbg = _get_program(flags_key, stage)
    in_maps = _make_in_maps(inputs, flags, cores)
    res = run_bass_kernel_spmd(nc, in_maps, cores)
    return res, dbg


def kernel(**inputs) -> np.ndarray:
    res, _ = _run(inputs)
    return np.stack([res.results[c]["out"] for c in range(NCORES)], axis=0)


def _timed_run(inputs, iters=10, stage="full"):
    """Run the kernel `iters` times and return (out [B,T,E], sec_per_iter)."""
    import time
    import jax
    from jax.experimental.shard_map import shard_map
    from jax.sharding import Mesh, PartitionSpec
    from concourse import mybir
    from concourse.bass2jax import (_bass_exec_p, install_neuronx_cc_hook,
                                    partition_id_tensor)

    install_neuronx_cc_hook()
    flags = _flags_for(inputs)
    flags_key = tuple(flags[k] for k in ("ln1_gb", "ln2_gb", "b_proj", "b2"))
    nc, _ = _get_program(flags_key, stage)
    in_maps = _make_in_maps(inputs, flags, list(range(NCORES)))

    partition_name = (nc.partition_id_tensor.name
                      if nc.partition_id_tensor else None)
    in_names, out_names, out_avals = [], [], []
    for alloc in nc.m.functions[0].allocations:
        if not isinstance(alloc, mybir.MemoryLocationSet):
            continue
        name = alloc.memorylocations[0].name
        if alloc.kind == "ExternalInput":
            if name != partition_name:
                in_names.append(name)
        elif alloc.kind == "ExternalOutput":
            out_names.append(name)
            shape = tuple(alloc.tensor_shape)
            dtype = mybir.dt.np(alloc.dtype)
            out_avals.append(jax.core.ShapedArray(shape, dtype))
    n_params = len(in_names)
    all_names = in_names + out_names
    if partition_name is not None:
        all_names = all_names + [partition_name]

    def _body(*args):
        operands = list(args)
        if partition_name is not None:
            operands.append(partition_id_tensor())
        outs = _bass_exec_p.bind(
            *operands,
            out_avals=tuple(out_avals),
            in_names=tuple(all_names),
            out_names=tuple(out_names),
            lowering_input_output_aliases=(),
            sim_require_finite=True,
            sim_require_nnan=True,
            nc=nc,
        )
        return tuple(outs)

    devices = jax.devices()[:NCORES]
    mesh = Mesh(np.asarray(devices), ("core",))
    n_outs = len(out_names)
    in_specs = (PartitionSpec("core"),) * (n_params + n_outs)
    out_specs = (PartitionSpec("core"),) * n_outs
    fn = jax.jit(shard_map(_body, mesh=mesh, in_specs=in_specs,
                           out_specs=out_specs, check_rep=False),
                 keep_unused=True)

    concat_in = [
        np.concatenate([np.asarray(in_maps[c][nm]) for c in range(NCORES)],
                       axis=0)
        for nm in in_names
    ]
    concat_zeros = [
        np.zeros((NCORES * a.shape[0], *a.shape[1:]), a.dtype)
        for a in out_avals
    ]
    dev_args = [jax.device_put(a) for a in concat_in + concat_zeros]
    out = fn(*dev_args)
    jax.block_until_ready(out)     # warm-up (compile + first run)
    per_call = []
    for _ in range(iters):
        t0 = time.perf_counter()
        out = fn(*dev_args)
        jax.block_until_ready(out)
        per_call.append(time.perf_counter() - t0)
    oi = out_names.index("out")
    res = np.asarray(out[oi]).reshape(NCORES, T, E)
    return res, min(per_call)


# revision 3
# speedup vs baseline: 1.4600x; 1.4600x over previous
"""Trainium2 Bass kernel v2 for a dense transformer block (pre-LN, causal MHA
+ GELU MLP).  Data-parallel over batch: B == 8 == n_cores, one batch element
per NeuronCore, no collectives.

What changed vs the v1 baseline (measured on this axon-tunneled TRN2 pod):
  - f32r matmuls run at fp32 rate (~850 ns per N=512 matmul, 4 cyc/col); all
    matmul operands are now bf16 (1 cyc/col), halving weight bytes too.
  - DMA cost here is dominated by a large fixed per-dma_start cost, so the
    305 DMAs of v1 are collapsed into ~15 big contiguous transfers of
    host-packed weights, split across the two HWDGE rings (sync + scalar).
  - Weights are loaded exactly once (v1 streamed w1/w2 twice).
  - Attention keeps per-head [64, T] tiles so the softmax-normalize multiply
    lands directly in the tile (v1 DMA'd the odd head across partitions).
  - Softmax denominator reciprocal is partition-broadcast with a K=1 matmul
    on the PE (v1 used 32 gpsimd SWDGE broadcast DMAs).
  - Causal masking uses subrange matmuls instead of zero-filling masked
    blocks (25% fewer score/attnV cycles).

Per-core layout strategy is otherwise the same as v1: LN in token layout via
bn_stats, PE-transpose to e-partition layout for the matmuls, attention
computed transposed with an appended ones-column on V producing the softmax
denominator, MLP split into four FF quarters so weights stream through SBUF.
"""

import numpy as np

B, T, E = 8, 1024, 1024
NH, HD, FF = 16, 64, 4096
NPAIR = NH // 2
EPS = 1e-5
NCORES = 8
TCH = T // 128           # 8 token chunks
ECH = E // 128           # 8 embedding chunks
FCH = FF // 128          # 32 mlp hidden chunks
FHALF = 4                # mlp FF chunks (weight streaming granularity)
FCH_H = FCH // FHALF     # 8 hidden chunks per streaming chunk
TQW = 512
NTQ = T // TQW           # 2

_STAGES = {"ln": 1, "qkv": 2, "attn": 3, "proj": 4, "full": 5}


def _build_program(flags, stage="full"):
    import concourse.bass as bass
    import concourse.tile as tile
    from concourse import bacc, mybir
    from concourse.masks import make_identity, make_upper_triangular

    sn = _STAGES[stage]
    f32 = mybir.dt.float32
    bf16 = mybir.dt.bfloat16
    AF = mybir.ActivationFunctionType

    nc = bacc.Bacc("TRN2", target_bir_lowering=False, debug=False,
                   num_devices=NCORES)

    x_d = nc.dram_tensor("x", [128, TCH, E], f32,
                         kind="ExternalInput").ap()
    wqT_d = nc.dram_tensor("wqT", [128, ECH, E], bf16,
                           kind="ExternalInput").ap()
    wkT_d = nc.dram_tensor("wkT", [128, ECH, E], bf16,
                           kind="ExternalInput").ap()
    wvT_d = nc.dram_tensor("wvT", [128, ECH, E], bf16,
                           kind="ExternalInput").ap()
    wpT_d = nc.dram_tensor("wpT", [HD, NH, E], bf16,
                           kind="ExternalInput").ap()
    w1T_d = nc.dram_tensor("w1T", [FHALF, 128, ECH, FF // FHALF], bf16,
                           kind="ExternalInput").ap()
    w2T_d = nc.dram_tensor("w2T", [FHALF, 128, FCH_H, E], bf16,
                           kind="ExternalInput").ap()
    b1c_d = nc.dram_tensor("b1c", [128, FCH], f32, kind="ExternalInput").ap()
    ln1g_d = ln1b_d = ln2g_d = ln2b_d = bp_d = b2_d = None
    if flags["ln1_gb"]:
        ln1g_d = nc.dram_tensor("ln1_g", [E], f32, kind="ExternalInput").ap()
        ln1b_d = nc.dram_tensor("ln1_b", [E], f32, kind="ExternalInput").ap()
    if flags["ln2_gb"]:
        ln2g_d = nc.dram_tensor("ln2_g", [E], f32, kind="ExternalInput").ap()
        ln2b_d = nc.dram_tensor("ln2_b", [E], f32, kind="ExternalInput").ap()
    if flags["b_proj"]:
        bp_d = nc.dram_tensor("b_proj", [E], f32, kind="ExternalInput").ap()
    if flags["b2"]:
        b2_d = nc.dram_tensor("b2", [E], f32, kind="ExternalInput").ap()
    # x/out are host-packed to [128, TCH, E] so the DMA is one descriptor
    # per partition (128 total) instead of 1024 strided ones.
    out_d = nc.dram_tensor("out", [128, TCH, E], f32,
                           kind="ExternalOutput").ap()

    dbg_outs = {}

    def dbg_tensor(name, shape):
        dbg_outs[name] = nc.dram_tensor(name, shape, f32,
                                        kind="ExternalOutput").ap()
        return dbg_outs[name]

    with tile.TileContext(nc) as tc:
        with (
            tc.tile_pool(name="const", bufs=1) as p_const,
            tc.tile_pool(name="resid", bufs=1) as p_resid,
            tc.tile_pool(name="attn", bufs=NH) as p_attn,
            tc.tile_pool(name="ps", bufs=7, space="PSUM") as p_ps,
        ):
            # ---- constants ----
            ident = p_const.tile([128, 128], bf16, tag="ident", name="ident")
            make_identity(nc, ident[:])
            tri_f = p_const.tile([128, 128], f32, tag="trif", name="trif")
            make_upper_triangular(nc, tri_f[:], val=1.0, diag=True)
            tri = p_const.tile([128, 128], bf16, tag="tri", name="tri")
            nc.vector.tensor_copy(tri[:], tri_f[:])
            epst = p_const.tile([128, 1], f32, tag="epst", name="epst")
            nc.vector.memset(epst[:], EPS)
            onesb = p_const.tile([128, HD], bf16, tag="onesb", name="onesb")
            nc.vector.memset(onesb[:], 1.0)
            b1c = p_const.tile([128, FCH], f32, tag="b1c", name="b1c")
            nc.scalar.dma_start(b1c[:], b1c_d[:, :])

            def bcast_row(dram_vec, tag):
                tf = p_const.tile([128, E], f32, tag=tag + "f", name=tag + "f")
                src = bass.AP(tensor=dram_vec.tensor, offset=dram_vec.offset,
                              ap=[[0, 128]] + list(dram_vec.ap))
                nc.gpsimd.dma_start(tf[:], src)
                tb = p_const.tile([128, E], bf16, tag=tag, name=tag)
                nc.vector.tensor_copy(tb[:], tf[:])
                return tf, tb

            ln1g_b = bcast_row(ln1g_d, "ln1g")[1] if flags["ln1_gb"] else None
            ln1b_b = bcast_row(ln1b_d, "ln1b")[1] if flags["ln1_gb"] else None
            ln2g_b = bcast_row(ln2g_d, "ln2g")[1] if flags["ln2_gb"] else None
            ln2b_b = bcast_row(ln2b_d, "ln2b")[1] if flags["ln2_gb"] else None
            bp_b = bcast_row(bp_d, "bpb")[0] if flags["b_proj"] else None
            b2_b = bcast_row(b2_d, "b2b")[0] if flags["b2"] else None

            # ---- residual: single [128, TCH, E] f32 tile, one DMA each way
            xt = p_resid.tile([128, TCH, E], f32, tag="resid", name="resid")
            nc.sync.dma_start(xt[:], x_d[:, :, :])

            # ---- layernorm in token layout + PE transpose to [E, T] ----
            def layer_norm_transposed(g_b, b_b, p_ht, p_htok, p_small):
                ht = [p_ht.tile([128, T], bf16, tag="ht", name="ht")
                      for _ in range(ECH)]
                for tch in range(TCH):
                    xti = xt[:, tch, :]
                    st = p_small.tile([128, 2, 6], f32, tag="st", name="st")
                    nc.vector.bn_stats(st[:, 0, :], xti[:, 0:512])
                    nc.vector.bn_stats(st[:, 1, :], xti[:, 512:1024])
                    mv = p_small.tile([128, 2], f32, tag="mv", name="mv")
                    nc.vector.bn_aggr(mv[:], st[:])
                    sq = p_small.tile([128, 1], f32, tag="sq", name="sq")
                    nc.scalar.activation(sq[:], mv[:, 1:2], AF.Sqrt,
                                         bias=epst[:])
                    rsig = p_small.tile([128, 1], f32, tag="rsig", name="rsig")
                    nc.vector.reciprocal(rsig[:], sq[:])
                    h = p_htok.tile([128, E], bf16, tag="htok", name="htok")
                    nc.vector.tensor_scalar(h[:], xti, mv[:, 0:1],
                                            rsig[:], mybir.AluOpType.subtract,
                                            mybir.AluOpType.mult)
                    if g_b is not None:
                        nc.vector.tensor_mul(h[:], h[:], g_b[:])
                        nc.vector.tensor_add(h[:], h[:], b_b[:])
                    for ec in range(ECH):
                        pst = p_ps.tile([128, 512], bf16, tag="ps", name="ps")
                        with nc.allow_low_precision(
                                reason="bf16 transpose of bf16 data"):
                            nc.tensor.transpose(pst[:, 0:128],
                                                h[:, 128 * ec:128 * (ec + 1)],
                                                ident[:])
                        nc.vector.tensor_copy(
                            ht[ec][:, 128 * tch:128 * (tch + 1)],
                            pst[:, 0:128])
                return ht

            # Pool enter order is LIFO-constrained: longest-lived first.
            # -- pool group B: vt + qk (closed after attention) --
            b_cms = [
                tc.tile_pool(name="vpool", bufs=TCH),
                tc.tile_pool(name="qk", bufs=4),
            ]
            p_v, p_qk = (cm.__enter__() for cm in b_cms)
            att_cms = [
                tc.tile_pool(name="esc", bufs=6),
                tc.tile_pool(name="norm", bufs=4),
            ]
            p_esc, p_norm = (cm.__enter__() for cm in att_cms)
            # -- pool group A: QKV weights + ht (closed after attention) --
            a_cms = [
                tc.tile_pool(name="wqkv", bufs=1),
                tc.tile_pool(name="ht", bufs=ECH),
                tc.tile_pool(name="htok", bufs=3),
                tc.tile_pool(name="small", bufs=6),
            ]
            p_w, p_ht, p_htok, p_small = (cm.__enter__() for cm in a_cms)

            # weight loads, split across the two HWDGE rings
            wvT_t = p_w.tile([128, ECH, E], bf16, tag="wv", name="wv")
            nc.sync.dma_start(wvT_t[:], wvT_d[:, :, :])
            wqT_t = p_w.tile([128, ECH, E], bf16, tag="wq", name="wq")
            nc.scalar.dma_start(wqT_t[:], wqT_d[:, :, :])
            wkT_t = p_w.tile([128, ECH, E], bf16, tag="wk", name="wk")
            nc.sync.dma_start(wkT_t[:], wkT_d[:, :, :])

            ht = layer_norm_transposed(ln1g_b, ln1b_b, p_ht, p_htok, p_small)

            if sn == 1:
                o = dbg_tensor("dbg_ht", [E, T])
                for ec in range(ECH):
                    ob = p_htok.tile([128, T], f32, tag="dbgcast",
                                     name="dbgcast")
                    nc.vector.tensor_copy(ob[:], ht[ec][:])
                    nc.sync.dma_start(o[128 * ec:128 * (ec + 1), :], ob[:])

            # ---- V = h @ wv -> token layout [t, head, 65] + ones col ----
            vt = []
            for tch in range(TCH):
                v = p_v.tile([128, NH, HD + 1], bf16, tag="v", name="v")
                nc.vector.memset(v[:, :, HD:HD + 1], 1.0)
                vt.append(v)
            for half in range(2):
                esl = slice(512 * half, 512 * (half + 1))
                for tch in range(TCH):
                    ps = p_ps.tile([128, 512], f32, tag="ps", name="ps")
                    for ec in range(ECH):
                        nc.tensor.matmul(
                            ps[:], ht[ec][:, 128 * tch:128 * (tch + 1)],
                            wvT_t[:, ec, esl], start=(ec == 0),
                            stop=(ec == ECH - 1))
                    nc.vector.tensor_copy(
                        vt[tch][:, 8 * half:8 * (half + 1), 0:HD],
                        ps[:].rearrange("p (h d) -> p h d", d=HD))

            # ---- per pair: Q/K, then attention for its two heads ----
            attn_h = [None] * NH
            for pair in range(NPAIR if sn >= 2 else 0):
                cols = slice(128 * pair, 128 * (pair + 1))
                qT = p_qk.tile([128, T], bf16, tag="qk", name="qk")
                kT = p_qk.tile([128, T], bf16, tag="qk", name="qk")
                for (w_t, dst) in ((wqT_t, qT), (wkT_t, kT)):
                    for th in range(NTQ):
                        tsl = slice(TQW * th, TQW * (th + 1))
                        ps = p_ps.tile([128, 512], f32, tag="ps", name="ps")
                        for ec in range(ECH):
                            nc.tensor.matmul(
                                ps[:], w_t[:, ec, cols], ht[ec][:, tsl],
                                start=(ec == 0), stop=(ec == ECH - 1))
                        nc.vector.tensor_copy(dst[:, tsl], ps[:])

                if sn == 2 and pair == 0:
                    oq = dbg_tensor("dbg_qT", [128, T])
                    qf = p_htok.tile([128, T], f32, tag="dbgq", name="dbgq")
                    nc.vector.tensor_copy(qf[:], qT[:])
                    nc.sync.dma_start(oq[:, :], qf[:])
                    ok_ = dbg_tensor("dbg_kT", [128, T])
                    kf = p_htok.tile([128, T], f32, tag="dbgq", name="dbgq")
                    nc.vector.tensor_copy(kf[:], kT[:])
                    nc.sync.dma_start(ok_[:, :], kf[:])
                    o2 = dbg_tensor("dbg_v", [T, NH * (HD + 1)])
                    for tch in range(TCH):
                        vf = p_htok.tile([128, NH * (HD + 1)], f32,
                                         tag="dbgv", name="dbgv")
                        nc.vector.tensor_copy(
                            vf[:], vt[tch][:].rearrange("p h d -> p (h d)"))
                        nc.sync.dma_start(o2[128 * tch:128 * (tch + 1), :],
                                          vf[:])
                    break

                if sn < 3:
                    break

                for hp in range(2):
                    h = 2 * pair + hp
                    rows = slice(HD * hp, HD * (hp + 1))
                    qh, kh = qT[rows, :], kT[rows, :]
                    att = p_attn.tile([HD, T], bf16, tag="attn", name="attn")
                    attn_h[h] = att
                    for bq in range(NTQ):
                        qsl = slice(TQW * bq, TQW * (bq + 1))
                        nbk = min(TCH, 4 * bq + 4)
                        ps_a = p_ps.tile([128, 512], f32, tag="ps", name="ps")
                        for bk in range(nbk):
                            d = bk - 4 * bq
                            col0 = 128 * d if d > 0 else 0
                            ps_s = p_ps.tile([128, 512], f32, tag="ps",
                                             name="ps")
                            nc.tensor.matmul(
                                ps_s[:, col0:512],
                                kh[:, 128 * bk:128 * (bk + 1)],
                                qh[:, TQW * bq + col0:TQW * (bq + 1)],
                                start=True, stop=True)
                            et = p_esc.tile([128, 512], bf16, tag="esc",
                                            name="esc")
                            nc.scalar.activation(et[:, col0:512],
                                                 ps_s[:, col0:512],
                                                 AF.Exp, scale=0.125)
                            if d >= 0:
                                dsl = slice(col0, col0 + 128)
                                nc.vector.tensor_mul(et[:, dsl], et[:, dsl],
                                                     tri[:])
                            nc.tensor.matmul(
                                ps_a[0:HD + 1, col0:512], vt[bk][:, h, :],
                                et[:, col0:512], start=(bk == 0),
                                stop=(bk == nbk - 1))
                        # normalize: rcp of denominator, PE-broadcast, mul
                        rcp = p_norm.tile([HD + 1, 512], bf16, tag="rcp",
                                          name="rcp")
                        with nc.allow_low_precision(
                                reason="bf16 softmax denom reciprocal"):
                            nc.vector.reciprocal(rcp[HD:HD + 1, :],
                                                 ps_a[HD:HD + 1, :])
                        ps_b = p_ps.tile([128, 512], f32, tag="ps", name="ps")
                        nc.tensor.matmul(ps_b[0:HD, :], onesb[HD:HD + 1, :],
                                         rcp[HD:HD + 1, :], start=True,
                                         stop=True)
                        bct = p_norm.tile([HD, 512], f32, tag="bct",
                                          name="bct")
                        nc.vector.tensor_copy(bct[:], ps_b[0:HD, :])
                        nc.vector.tensor_mul(att[:, qsl], ps_a[0:HD, :],
                                             bct[:])

            # free QKV weights, ht, qk, vt, attention transients
            for cm in reversed(a_cms):
                cm.__exit__(None, None, None)
            for cm in reversed(att_cms):
                cm.__exit__(None, None, None)
            for cm in reversed(b_cms):
                cm.__exit__(None, None, None)

            if sn == 3:
                o = dbg_tensor("dbg_attnT", [NH * HD, T])
                with tc.tile_pool(name="dbga", bufs=2) as p_dbg:
                    for h in range(NH):
                        af = p_dbg.tile([HD, T], f32, tag="af", name="af")
                        nc.vector.tensor_copy(af[:], attn_h[h][:])
                        nc.sync.dma_start(o[HD * h:HD * (h + 1), :], af[:])

            # MLP + proj weight pools (w1/w2 stream in quarters; wp on top
            # of the stack so it can be released right after proj)
            w1_cm = tc.tile_pool(name="w1", bufs=2)
            p_w1 = w1_cm.__enter__()
            w2_cm = tc.tile_pool(name="w2", bufs=2)
            p_w2 = w2_cm.__enter__()
            wp_cm = tc.tile_pool(name="wp", bufs=1)
            p_wp = wp_cm.__enter__()
            wpT_t = p_wp.tile([HD, NH, E], bf16, tag="wp", name="wp")
            nc.scalar.dma_start(wpT_t[:], wpT_d[:, :, :])

            def load_w1(fh):
                t = p_w1.tile([128, ECH, FF // FHALF], bf16, tag="w1",
                              name="w1")
                (nc.sync if fh % 2 == 0 else nc.scalar).dma_start(
                    t[:], w1T_d[fh, :, :, :])
                return t

            def load_w2(fh):
                t = p_w2.tile([128, FCH_H, E], bf16, tag="w2", name="w2")
                (nc.scalar if fh % 2 == 0 else nc.sync).dma_start(
                    t[:], w2T_d[fh, :, :, :])
                return t

            w1h = [load_w1(0)]
            w2h = [load_w2(0)]

            # ---- out = x + attn @ w_proj (+ b_proj), in-place xt ----
            if sn >= 4:
                for eo in range(2):
                    esl = slice(512 * eo, 512 * (eo + 1))
                    for tch in range(TCH):
                        ps = p_ps.tile([128, 512], f32, tag="ps", name="ps")
                        for h in range(NH):
                            nc.tensor.matmul(
                                ps[:],
                                attn_h[h][:, 128 * tch:128 * (tch + 1)],
                                wpT_t[:, h, esl], start=(h == 0),
                                stop=(h == NH - 1))
                        nc.vector.tensor_add(xt[:, tch, esl], ps[:],
                                             xt[:, tch, esl])
                        if flags["b_proj"]:
                            nc.vector.tensor_add(xt[:, tch, esl],
                                                 xt[:, tch, esl],
                                                 bp_b[:, esl])

            wp_cm.__exit__(None, None, None)

            if sn == 4:
                o = dbg_tensor("dbg_out1", [T, E])
                nc.sync.dma_start(o.rearrange("(c p) e -> p c e", p=128),
                                  xt[:])

            if sn >= 5:
                # ---- LN2 + transpose ----
                mlp_cms = [
                    tc.tile_pool(name="h2t", bufs=ECH),
                    tc.tile_pool(name="htok2", bufs=3),
                    tc.tile_pool(name="small2", bufs=6),
                    tc.tile_pool(name="gt", bufs=FCH_H),
                ]
                p_h2t, p_htok2, p_small2, p_gt = (
                    cm.__enter__() for cm in mlp_cms)
                h2t = layer_norm_transposed(ln2g_b, ln2b_b, p_h2t, p_htok2,
                                            p_small2)

                # ---- MLP: u = w1^T h2T; g = gelu(u+b1); xt += g^T w2 ----
                for fh in range(FHALF):
                    if fh + 1 < FHALF:
                        w1h.append(load_w1(fh + 1))
                        w2h.append(load_w2(fh + 1))
                    gt = []
                    for fl in range(FCH_H):
                        fc = FCH_H * fh + fl
                        g = p_gt.tile([128, T], bf16, tag="gt", name="gt")
                        gt.append(g)
                        for th in range(NTQ):
                            tsl = slice(TQW * th, TQW * (th + 1))
                            ps = p_ps.tile([128, 512], f32, tag="ps",
                                           name="ps")
                            for ec in range(ECH):
                                nc.tensor.matmul(
                                    ps[:],
                                    w1h[fh][:, ec, 128 * fl:128 * (fl + 1)],
                                    h2t[ec][:, tsl], start=(ec == 0),
                                    stop=(ec == ECH - 1))
                            nc.scalar.activation(g[:, tsl], ps[:], AF.Gelu,
                                                 bias=b1c[:, fc:fc + 1])
                    for eo in range(2):
                        esl = slice(512 * eo, 512 * (eo + 1))
                        for tl in range(TCH):
                            ps = p_ps.tile([128, 512], f32, tag="ps",
                                           name="ps")
                            for fl in range(FCH_H):
                                nc.tensor.matmul(
                                    ps[:],
                                    gt[fl][:, 128 * tl:128 * (tl + 1)],
                                    w2h[fh][:, fl, esl], start=(fl == 0),
                                    stop=(fl == FCH_H - 1))
                            nc.vector.tensor_add(xt[:, tl, esl], ps[:],
                                                 xt[:, tl, esl])
                            if flags["b2"] and fh == FHALF - 1:
                                nc.vector.tensor_add(xt[:, tl, esl],
                                                     xt[:, tl, esl],
                                                     b2_b[:, esl])
                for cm in reversed(mlp_cms):
                    cm.__exit__(None, None, None)

                nc.scalar.dma_start(out_d[:, :, :], xt[:])

            w2_cm.__exit__(None, None, None)
            w1_cm.__exit__(None, None, None)

    nc.compile()
    return nc, dbg_outs


_CACHE = {}


def _get_program(flags_key, stage="full"):
    key = (flags_key, stage)
    if key not in _CACHE:
        flags = dict(zip(("ln1_gb", "ln2_gb", "b_proj", "b2"), flags_key))
        _CACHE[key] = _build_program(flags, stage)
    return _CACHE[key]


def _flags_for(inputs):
    return {
        "ln1_gb": not (np.all(np.asarray(inputs["ln1_g"]) == 1.0)
                       and np.all(np.asarray(inputs["ln1_b"]) == 0.0)),
        "ln2_gb": not (np.all(np.asarray(inputs["ln2_g"]) == 1.0)
                       and np.all(np.asarray(inputs["ln2_b"]) == 0.0)),
        "b_proj": not np.all(np.asarray(inputs["b_proj"]) == 0.0),
        "b2": not np.all(np.asarray(inputs["b2"]) == 0.0),
    }


def _pack_weights(inputs, flags):
    import ml_dtypes
    bf16 = ml_dtypes.bfloat16
    f32 = np.float32

    def a(name):
        return np.asarray(inputs[name], f32)

    common = {}
    common["wqT"] = np.ascontiguousarray(
        a("wq").reshape(ECH, 128, E).transpose(1, 0, 2)).astype(bf16)
    common["wkT"] = np.ascontiguousarray(
        a("wk").reshape(ECH, 128, E).transpose(1, 0, 2)).astype(bf16)
    common["wvT"] = np.ascontiguousarray(
        a("wv").reshape(ECH, 128, E).transpose(1, 0, 2)).astype(bf16)
    common["wpT"] = np.ascontiguousarray(
        a("w_proj").reshape(NH, HD, E).transpose(1, 0, 2)).astype(bf16)
    w1 = a("w1")
    fw = FF // FHALF
    common["w1T"] = np.ascontiguousarray(np.stack([
        w1[:, fh * fw:(fh + 1) * fw].reshape(ECH, 128, fw).transpose(1, 0, 2)
        for fh in range(FHALF)])).astype(bf16)
    w2 = a("w2")
    common["w2T"] = np.ascontiguousarray(np.stack([
        w2[fh * fw:(fh + 1) * fw, :].reshape(FCH_H, 128, E).transpose(1, 0, 2)
        for fh in range(FHALF)])).astype(bf16)
    common["b1c"] = np.ascontiguousarray(
        a("b1").reshape(FCH, 128).transpose(1, 0)).astype(f32)
    for name, flag in (("ln1_g", "ln1_gb"), ("ln1_b", "ln1_gb"),
                       ("ln2_g", "ln2_gb"), ("ln2_b", "ln2_gb"),
                       ("b_proj", "b_proj"), ("b2", "b2")):
        if flags[flag]:
            common[name] = np.ascontiguousarray(inputs[name], f32)
    return common


def _pack_x(xc):
    return np.ascontiguousarray(
        np.asarray(xc, np.float32).reshape(TCH, 128, E).transpose(1, 0, 2))


def _unpack_out(o):
    return np.asarray(o).transpose(1, 0, 2).reshape(T, E)


def _make_in_maps(inputs, flags, cores):
    common = _pack_weights(inputs, flags)
    x = np.asarray(inputs["x"], np.float32)
    return [{"x": _pack_x(x[c]), **common} for c in cores]


def _run(inputs, stage="full", cores=None):
    from concourse.bass_utils import run_bass_kernel_spmd

    if cores is None:
        cores = list(range(NCORES))
    flags = _flags_for(inputs)
    flags_key = tuple(flags[k] for k in ("ln1_gb", "ln2_gb", "b_proj", "b2"))
    nc, dbg = _get_program(flags_key, stage)
    in_maps = _make_in_maps(inputs, flags, cores)
    res = run_bass_kernel_spmd(nc, in_maps, cores)
    return res, dbg


def kernel(**inputs) -> np.ndarray:
    res, _ = _run(inputs)
    return np.stack([_unpack_out(res.results[c]["out"])
                     for c in range(NCORES)], axis=0)


def _timed_run(inputs, iters=10, stage="full"):
    """Run the kernel `iters` times and return (out [B,T,E], sec_per_iter)."""
    import time
    import jax
    from jax.experimental.shard_map import shard_map
    from jax.sharding import Mesh, PartitionSpec
    from concourse import mybir
    from concourse.bass2jax import (_bass_exec_p, install_neuronx_cc_hook,
                                    partition_id_tensor)

    install_neuronx_cc_hook()
    flags = _flags_for(inputs)
    flags_key = tuple(flags[k] for k in ("ln1_gb", "ln2_gb", "b_proj", "b2"))
    nc, _ = _get_program(flags_key, stage)
    in_maps = _make_in_maps(inputs, flags, list(range(NCORES)))

    partition_name = (nc.partition_id_tensor.name
                      if nc.partition_id_tensor else None)
    in_names, out_names, out_avals = [], [], []
    for alloc in nc.m.functions[0].allocations:
        if not isinstance(alloc, mybir.MemoryLocationSet):
            continue
        name = alloc.memorylocations[0].name
        if alloc.kind == "ExternalInput":
            if name != partition_name:
                in_names.append(name)
        elif alloc.kind == "ExternalOutput":
            out_names.append(name)
            shape = tuple(alloc.tensor_shape)
            dtype = mybir.dt.np(alloc.dtype)
            out_avals.append(jax.core.ShapedArray(shape, dtype))
    n_params = len(in_names)
    all_names = in_names + out_names
    if partition_name is not None:
        all_names = all_names + [partition_name]

    def _body(*args):
        operands = list(args)
        if partition_name is not None:
            operands.append(partition_id_tensor())
        outs = _bass_exec_p.bind(
            *operands,
            out_avals=tuple(out_avals),
            in_names=tuple(all_names),
            out_names=tuple(out_names),
            lowering_input_output_aliases=(),
            sim_require_finite=True,
            sim_require_nnan=True,
            nc=nc,
        )
        return tuple(outs)

    devices = jax.devices()[:NCORES]
    mesh = Mesh(np.asarray(devices), ("core",))
    n_outs = len(out_names)
    in_specs = (PartitionSpec("core"),) * (n_params + n_outs)
    out_specs = (PartitionSpec("core"),) * n_outs
    fn = jax.jit(shard_map(_body, mesh=mesh, in_specs=in_specs,
                           out_specs=out_specs, check_rep=False),
                 keep_unused=True)

    concat_in = [
        np.concatenate([np.asarray(in_maps[c][nm]) for c in range(NCORES)],
                       axis=0)
        for nm in in_names
    ]
    concat_zeros = [
        np.zeros((NCORES * a.shape[0], *a.shape[1:]), a.dtype)
        for a in out_avals
    ]
    dev_args = [jax.device_put(a) for a in concat_in + concat_zeros]
    out = fn(*dev_args)
    jax.block_until_ready(out)     # warm-up (compile + first run)
    per_call = []
    for _ in range(iters):
        t0 = time.perf_counter()
        out = fn(*dev_args)
        jax.block_until_ready(out)
        per_call.append(time.perf_counter() - t0)
    oi = out_names.index("out")
    res = np.asarray(out[oi]).reshape(NCORES, 128, TCH, E)
    res = res.transpose(0, 2, 1, 3).reshape(NCORES, T, E)
    return res, min(per_call)
